# revision 1
# baseline (speedup 1.0000x reference)
"""GATv2 layer on 8 Trainium2 NeuronCores (Bass/Tile).

Strategy (target-major, fully static SPMD):
  * Host relabels nodes (degree-snake) so each 128-node window has ~equal
    incoming-edge mass, sorts edges by (relabeled) target, groups them into
    fixed 128-node windows, and splits each window's edges by source half
    (node id < NH) so gather indices fit int16 for dma_gather.
  * Node projections (left/right/values) are computed on device, sharded
    over cores (each core projects its 6272-node slice); the packed
    right||values table is AllGathered; `left` stays core-local in SBUF.
  * Per window: two dma_gather calls fetch right||values rows per edge;
    `left[target]` is expanded via one-hot matmul from the SBUF left table;
    scores/softmax-numerator/denominator are computed per edge and
    segment-reduced into PSUM via one-hot matmuls (all edges of a target
    live in one window => no cross-core reduction needed).
  * Output = (num/den) @ Wo + bo, written transposed; host undoes layout.

One-hot matrices are precomputed on host and streamed (HWDGE sequential) —
cheaper than descriptor-bound compute paths on device.
"""
import numpy as np
import ml_dtypes

N_CORES = 8
N_NODES = 50000
NPAD = 50176          # 8 * 6272
PER_CORE = NPAD // N_CORES   # 6272
WIN = 128             # nodes per window
W_PER_CORE = PER_CORE // WIN  # 49
W_GLOBAL = NPAD // WIN        # 392
NH = NPAD // 2        # half-table rows (25088 < 32768 -> int16 ok)
IN_F = 256
OUT_F = 128
E_F = 64
H = 8
HD = 16
NEG_SLOPE = 0.2
bf16 = ml_dtypes.bfloat16


# ----------------------------------------------------------------------------
# host-side graph restructuring
# ----------------------------------------------------------------------------

def _host_prepare(node_features, edge_index, edge_features, Wl, bl, Wr, br,
                  We, be, attn_vector, Wv, bv):
    s = np.asarray(edge_index[0], dtype=np.int64)
    t = np.asarray(edge_index[1], dtype=np.int64)
    E = s.shape[0]

    # --- degree-snake relabeling: balance incoming-edge mass per 128-window
    deg = np.bincount(t, minlength=NPAD).astype(np.int64)
    order = np.argsort(-deg, kind="stable")  # all NPAD ids (pads have deg 0)
    order = order[order < N_NODES] if N_NODES < NPAD else order
    # snake across W_GLOBAL windows
    new_id = np.empty(NPAD, dtype=np.int64)
    filln = np.zeros(W_GLOBAL, dtype=np.int64)
    w_seq = np.arange(len(order)) % (2 * W_GLOBAL)
    w_seq = np.where(w_seq < W_GLOBAL, w_seq, 2 * W_GLOBAL - 1 - w_seq)
    for node, w in zip(order, w_seq):
        new_id[node] = w * WIN + filln[w]
        filln[w] += 1
    # pad ids fill remaining slots
    spare = []
    for w in range(W_GLOBAL):
        for k in range(filln[w], WIN):
            spare.append(w * WIN + k)
    spare = np.array(spare, dtype=np.int64)
    pad_nodes = np.arange(N_NODES, NPAD)
    new_id[pad_nodes] = spare[:len(pad_nodes)] if len(pad_nodes) else spare[:0]
    # note: if fewer spare than pads something is off
    inv_id = np.empty(NPAD, dtype=np.int64)
    inv_id[new_id] = np.arange(NPAD)

    ns = new_id[s]
    nt = new_id[t]

    # --- group edges by (window, source-half)
    w_of_edge = nt // WIN
    h_of_edge = (ns >= NH).astype(np.int64)
    key = w_of_edge * 2 + h_of_edge
    eorder = np.argsort(key, kind="stable")
    key_s = key[eorder]
    # counts per (window, half)
    cnt = np.bincount(key_s, minlength=2 * W_GLOBAL).reshape(W_GLOBAL, 2)
    D = int(np.ceil(cnt.max() / WIN))
    NI = D * WIN                 # slots per half
    SLOTS_W = 2 * NI             # slots per window
    EPC = W_PER_CORE * SLOTS_W   # padded edge slots per core

    # slot assignment for each sorted edge
    starts = np.zeros(2 * W_GLOBAL + 1, dtype=np.int64)
    np.cumsum(cnt.reshape(-1), out=starts[1:])
    within = np.arange(E, dtype=np.int64) - starts[key_s]
    slot_global = key_s // 2 * SLOTS_W + (key_s % 2) * NI + within

    # staged per-slot arrays (global, then reshaped per core)
    TOT = W_GLOBAL * SLOTS_W
    tlw_slot = np.full(TOT, 200, dtype=np.int32)
    src_slot = np.zeros(TOT, dtype=np.int64)
    ef_slot = np.zeros((TOT, E_F), dtype=np.float32)
    es = eorder
    tlw_slot[slot_global] = (nt[es] % WIN).astype(np.int32)
    src_slot[slot_global] = ns[es] % NH
    ef_slot[slot_global] = np.asarray(edge_features, dtype=np.float32)[es]

    C = 2 * D  # chunks per window

    # one-hot streams, layout [W_GLOBAL, 128(part), C*128] partition-contiguous
    tlw_wcp = tlw_slot.reshape(W_GLOBAL, C, WIN)  # [w, chunk, pos]
    n_ar = np.arange(WIN, dtype=np.int32)
    # onehotE[w, p, c, n] = (tlw[w, c, p] == n)
    ohE = (tlw_wcp.transpose(0, 2, 1)[:, :, :, None] == n_ar[None, None, None, :])
    ohE = ohE.astype(bf16).reshape(W_GLOBAL, WIN, C * WIN)
    # onehotT[w, n, c, e] = (tlw[w, c, e] == n)
    ohT = (n_ar[None, :, None, None] == tlw_wcp[:, None, :, :])
    ohT = ohT.astype(bf16).reshape(W_GLOBAL, WIN, C * WIN)

    # edge features transposed + ones row: [W_GLOBAL, 65, C*128]
    efT = np.concatenate([ef_slot, np.ones((TOT, 1), np.float32)], axis=1)
    efT = efT.reshape(W_GLOBAL, C * WIN, E_F + 1).transpose(0, 2, 1)
    efT = np.ascontiguousarray(efT, dtype=np.float32).astype(bf16)

    # int16 gather indices, wrapped in 16 partitions replicated x8:
    # position i in a half -> idxs[[i%16, i//16]]
    src_wh = src_slot.reshape(W_GLOBAL, 2, NI)
    wrap = src_wh.reshape(W_GLOBAL, 2, NI // 16, 16).transpose(0, 1, 3, 2)
    wrap = wrap.reshape(W_GLOBAL, 2, 16, NI // 16).astype(np.int16)
    src16 = np.tile(wrap, (1, 1, 8, 1)).reshape(W_GLOBAL, 2, 128, NI // 16)
    src16 = np.ascontiguousarray(src16.transpose(0, 2, 1, 3)).reshape(
        W_GLOBAL, 128, 2 * (NI // 16))

    # node features (relabeled, transposed, +ones row, padded to 384 rows)
    nf = np.zeros((NPAD, IN_F), dtype=np.float32)
    nf[new_id[:N_NODES]] = np.asarray(node_features, dtype=np.float32)
    nfT = np.zeros((384, NPAD), dtype=np.float32)
    nfT[:IN_F] = nf.T
    nfT[IN_F] = 1.0
    nfT = nfT.astype(bf16)

    # weights
    def aug(Wm, bv_):
        a = np.zeros((384, Wm.shape[1]), dtype=np.float32)
        a[:IN_F] = np.asarray(Wm, dtype=np.float32)
        a[IN_F] = np.asarray(bv_, dtype=np.float32)
        return a
    Wrv = np.concatenate([aug(Wr, br), aug(Wv, bv)], axis=1).astype(bf16)  # [384, 256]
    Wla = aug(Wl, bl).astype(bf16)                                          # [384, 128]
    Wea = np.zeros((E_F + 1, H), dtype=np.float32)
    Wea[:E_F] = np.asarray(We, dtype=np.float32)
    Wea[E_F] = np.asarray(be, dtype=np.float32)
    Wea = Wea.astype(bf16)
    attn_flat = np.asarray(attn_vector, dtype=np.float32).reshape(-1)  # [128]
    attn_mat = np.tile(attn_flat[None, :], (128, C)).astype(bf16)

    host = dict(D=D, NI=NI, C=C, EPC=EPC, inv_id=inv_id, new_id=new_id)
    per_core = []
    for c in range(N_CORES):
        wlo, whi = c * W_PER_CORE, (c + 1) * W_PER_CORE
        per_core.append({
            "nfT": np.ascontiguousarray(nfT[:, c * PER_CORE:(c + 1) * PER_CORE]),
            "ohE": np.ascontiguousarray(ohE[wlo:whi]),
            "ohT": np.ascontiguousarray(ohT[wlo:whi]),
            "efT": np.ascontiguousarray(efT[wlo:whi]),
            "src16": np.ascontiguousarray(src16[wlo:whi]),
            "Wrv": Wrv, "Wla": Wla, "Wea": Wea, "attn": attn_mat,
        })
    return host, per_core


# ----------------------------------------------------------------------------
# device kernel
# ----------------------------------------------------------------------------

def _build_nc(D):
    import concourse.bass as bass
    import concourse.bacc as bacc
    import concourse.tile as tile
    from concourse import mybir
    from concourse.masks import make_identity

    f32 = mybir.dt.float32
    b16 = mybir.dt.bfloat16
    i16 = mybir.dt.int16
    NI = D * WIN
    C = 2 * D
    CW = C * WIN

    nc = bacc.Bacc("TRN2", num_devices=N_CORES, debug=False)
    d_nfT = nc.dram_tensor("nfT", [384, PER_CORE], b16, kind="ExternalInput").ap()
    d_ohE = nc.dram_tensor("ohE", [W_PER_CORE, 128, CW], b16, kind="ExternalInput").ap()
    d_ohT = nc.dram_tensor("ohT", [W_PER_CORE, 128, CW], b16, kind="ExternalInput").ap()
    d_efT = nc.dram_tensor("efT", [W_PER_CORE, E_F + 1, CW], b16, kind="ExternalInput").ap()
    d_src = nc.dram_tensor("src16", [W_PER_CORE, 128, 2 * (NI // 16)], i16, kind="ExternalInput").ap()
    d_Wrv = nc.dram_tensor("Wrv", [384, 256], b16, kind="ExternalInput").ap()
    d_Wla = nc.dram_tensor("Wla", [384, 128], b16, kind="ExternalInput").ap()
    d_Wea = nc.dram_tensor("Wea", [E_F + 1, H], b16, kind="ExternalInput").ap()
    d_attn = nc.dram_tensor("attn", [128, CW], b16, kind="ExternalInput").ap()
    d_Wo = nc.dram_tensor("Wo", [128, 128], b16, kind="ExternalInput").ap()
    d_bo = nc.dram_tensor("bo", [128, 1], f32, kind="ExternalInput").ap()
    d_out = nc.dram_tensor("outT", [128, PER_CORE], f32, kind="ExternalOutput").ap()

    with tile.TileContext(nc) as tc:
        with (
            tc.tile_pool(name="const", bufs=1) as cons,
            tc.tile_pool(name="tbl", bufs=3) as tblp,
            tc.tile_pool(name="win", bufs=4) as winp,
            tc.tile_pool(name="psum", bufs=2, space="PSUM") as psp,
            tc.tile_pool(name="dram", bufs=1, space="DRAM") as dram,
        ):
            # ---- constants
            Wrv_sb = cons.tile([128, 3, 256], b16)
            nc.sync.dma_start(out=Wrv_sb[:], in_=d_Wrv.rearrange("(j p) n -> p j n", p=128))
            Wla_sb = cons.tile([128, 3, 128], b16)
            nc.sync.dma_start(out=Wla_sb[:], in_=d_Wla.rearrange("(j p) n -> p j n", p=128))
            Wea_sb = cons.tile([E_F + 1, H], b16)
            nc.sync.dma_start(out=Wea_sb[:], in_=d_Wea[:, :])
            attn_sb = cons.tile([128, CW], b16)
            nc.sync.dma_start(out=attn_sb[:], in_=d_attn[:, :])
            Wo_sb = cons.tile([128, 128], b16)
            nc.sync.dma_start(out=Wo_sb[:], in_=d_Wo[:, :])
            bo_sb = cons.tile([128, 1], f32)
            nc.sync.dma_start(out=bo_sb[:], in_=d_bo[:, :])
            ident = cons.tile([128, 128], b16)
            make_identity(nc, ident[:])
            left_tab = cons.tile([128, W_PER_CORE * 128], b16)

            # ---- table phase: project this core's node slice
            # (KREPS>1 replicates the whole kernel body for slope-based timing)
            import os
            _kreps = int(os.environ.get("KREPS", "1"))
            rv_loc = dram.tile([PER_CORE, 256], b16)
            rv_full = dram.tile([NPAD, 256], b16)
          # replication loop (timing only; KREPS=1 in production)
          # fmt: off
            for _rep in range(_kreps):
              for tti in range(W_PER_CORE):
                nf3 = tblp.tile([128, 3, 128], b16, tag="nf3")
                nc.sync.dma_start(
                    out=nf3[:],
                    in_=d_nfT.rearrange("(j p) n -> p j n", p=128)[:, :, tti * 128:(tti + 1) * 128])
                ps_rv = psp.tile([128, 256], f32, tag="pA")
                ps_l = psp.tile([128, 128], f32, tag="pB")
                for j in range(3):
                    nc.tensor.matmul(out=ps_rv[:], lhsT=nf3[:, j, :], rhs=Wrv_sb[:, j, :],
                                     start=(j == 0), stop=(j == 2))
                for j in range(3):
                    nc.tensor.matmul(out=ps_l[:], lhsT=nf3[:, j, :], rhs=Wla_sb[:, j, :],
                                     start=(j == 0), stop=(j == 2))
                rv_sb = tblp.tile([128, 256], b16, tag="rvsb")
                nc.vector.tensor_copy(out=rv_sb[:], in_=ps_rv[:])
                nc.vector.tensor_copy(out=left_tab[:, tti * 128:(tti + 1) * 128], in_=ps_l[:])
                nc.sync.dma_start(out=rv_loc[tti * 128:(tti + 1) * 128, :], in_=rv_sb[:])

            nc.gpsimd.collective_compute(
                "AllGather", mybir.AluOpType.bypass,
                replica_groups=[list(range(N_CORES))],
                ins=[rv_loc[:].opt()], outs=[rv_full[:].opt()],
            )

            # ---- edge phase
            import os
            _kreps = int(os.environ.get("KREPS", "1"))
            for _rep in range(_kreps):
              for w in range(W_PER_CORE):
                ohE_sb = winp.tile([128, CW], b16, tag="ohE")
                nc.sync.dma_start(out=ohE_sb[:], in_=d_ohE[w, :, :])
                ohT_sb = winp.tile([128, CW], b16, tag="ohT")
                nc.sync.dma_start(out=ohT_sb[:], in_=d_ohT[w, :, :])
                efT_sb = winp.tile([E_F + 1, CW], b16, tag="efT")
                nc.sync.dma_start(out=efT_sb[:], in_=d_efT[w, :, :])
                src_sb = winp.tile([128, 2 * (NI // 16)], i16, tag="src")
                nc.sync.dma_start(out=src_sb[:], in_=d_src[w, :, :])

                rv_g = winp.tile([128, C, 256], b16, tag="rvg")
                nc.gpsimd.dma_gather(
                    out_ap=rv_g[:, :D, :], in_ap=rv_full[:NH, :],
                    idxs_ap=src_sb[:, :NI // 16],
                    num_idxs=NI, num_idxs_reg=NI, elem_size=256, single_packet=False)
                nc.gpsimd.dma_gather(
                    out_ap=rv_g[:, D:, :], in_ap=rv_full[NH:, :],
                    idxs_ap=src_sb[:, NI // 16:],
                    num_idxs=NI, num_idxs_reg=NI, elem_size=256, single_packet=False)

                act_win = winp.tile([128, C, 128], b16, tag="act")
                ps_bias = psp.tile([128, C * H], f32, tag="pB")
                for cc in range(C):
                    ps_comb = psp.tile([128, 128], f32, tag="pA")
                    nc.tensor.matmul(out=ps_comb[:], lhsT=ohT_sb[:, cc * 128:(cc + 1) * 128],
                                     rhs=left_tab[:, w * 128:(w + 1) * 128],
                                     start=True, stop=False)
                    nc.tensor.matmul(out=ps_comb[:], lhsT=ident[:], rhs=rv_g[:, cc, 0:128],
                                     start=False, stop=True)
                    nc.scalar.activation(out=act_win[:, cc, :], in_=ps_comb[:],
                                         func=mybir.ActivationFunctionType.Lrelu,
                                         alpha=NEG_SLOPE)
                    nc.tensor.matmul(out=ps_bias[:, cc * H:(cc + 1) * H],
                                     lhsT=efT_sb[:, cc * 128:(cc + 1) * 128],
                                     rhs=Wea_sb[:], start=True, stop=True)

                prod = winp.tile([128, C, 128], b16, tag="prod")
                nc.vector.tensor_tensor(
                    out=prod[:].rearrange("p c f -> p (c f)"), in0=act_win[:].rearrange("p c f -> p (c f)"),
                    in1=attn_sb[:], op=mybir.AluOpType.mult)
                scores = winp.tile([128, C * H], f32, tag="scores")
                nc.vector.tensor_reduce(
                    out=scores[:], in_=prod[:].rearrange("p c (h d) -> p c h d", h=H),
                    axis=mybir.AxisListType.X, op=mybir.AluOpType.add)
                scores2 = winp.tile([128, C * H], f32, tag="scores2")
                nc.vector.tensor_tensor(out=scores2[:], in0=scores[:], in1=ps_bias[:],
                                        op=mybir.AluOpType.add)
                exp_sb = winp.tile([128, C, H], b16, tag="exp")
                nc.scalar.activation(out=exp_sb[:], in_=scores2[:].rearrange("p (c h) -> p c h", h=H),
                                     func=mybir.ActivationFunctionType.Exp)

                wgt = winp.tile([128, C, 136], b16, tag="wgt")
                nc.vector.tensor_copy(out=wgt[:, :, 128:136], in_=exp_sb[:])
                exp_ap = exp_sb[:]
                nc.vector.tensor_tensor(
                    out=wgt[:, :, 0:128].rearrange("p c (h d) -> p c h d", h=H),
                    in0=rv_g[:, :, 128:256].rearrange("p c (h d) -> p c h d", h=H),
                    in1=bass.AP(tensor=exp_ap.tensor, offset=exp_ap.offset,
                                ap=[[exp_ap.ap[0][0], 128],
                                    [exp_ap.ap[1][0], C], [1, H], [0, HD]]),
                    op=mybir.AluOpType.mult)

                ps_agg = psp.tile([128, 136], f32, tag="pC")
                for cc in range(C):
                    nc.tensor.matmul(out=ps_agg[:], lhsT=ohE_sb[:, cc * 128:(cc + 1) * 128],
                                     rhs=wgt[:, cc, :], start=(cc == 0), stop=(cc == C - 1))

                # ---- finalize window: out = (num/den) @ Wo + bo (transposed)
                den = winp.tile([128, H], f32, tag="den")
                nc.vector.tensor_scalar_add(out=den[:], in0=ps_agg[:, 128:136], scalar1=1e-10)
                rec = winp.tile([128, H], f32, tag="rec")
                nc.vector.reciprocal(out=rec[:], in_=den[:])
                h_sb = winp.tile([128, 128], b16, tag="hsb")
                rec_ap = rec[:]
                nc.vector.tensor_tensor(
                    out=h_sb[:].rearrange("p (h d) -> p h d", h=H),
                    in0=ps_agg[:, 0:128].rearrange("p (h d) -> p h d", h=H),
                    in1=bass.AP(tensor=rec_ap.tensor, offset=rec_ap.offset,
                                ap=[[rec_ap.ap[0][0], 128], [1, H], [0, HD]]),
                    op=mybir.AluOpType.mult)
                ps_T = psp.tile([128, 128], b16, tag="pD")
                nc.tensor.transpose(out=ps_T[:], in_=h_sb[:], identity=ident[:])
                hT_sb = winp.tile([128, 128], b16, tag="hTsb")
                nc.vector.tensor_copy(out=hT_sb[:], in_=ps_T[:])
                ps_out = psp.tile([128, 128], f32, tag="pD")
                nc.tensor.matmul(out=ps_out[:], lhsT=Wo_sb[:], rhs=hT_sb[:],
                                 start=True, stop=True)
                out_sb = winp.tile([128, 128], f32, tag="osb")
                nc.scalar.activation(out=out_sb[:], in_=ps_out[:],
                                     func=mybir.ActivationFunctionType.Identity,
                                     bias=bo_sb[:])
                nc.sync.dma_start(out=d_out[:, w * 128:(w + 1) * 128], in_=out_sb[:])
    nc.compile()
    return nc


# ----------------------------------------------------------------------------
# inline SPMD runner (self-contained; mirrors concourse.bass2jax.run_bass_via_pjrt)
# ----------------------------------------------------------------------------

def _run_spmd(nc, in_maps):
    import jax
    import numpy as _np
    from jax.sharding import Mesh, PartitionSpec
    from jax.experimental.shard_map import shard_map
    import concourse.mybir as mybir
    from concourse.bass2jax import install_neuronx_cc_hook, _bass_exec_p, partition_id_tensor

    install_neuronx_cc_hook()
    partition_name = nc.partition_id_tensor.name if nc.partition_id_tensor else None
    in_names, out_names, out_avals, zero_outs = [], [], [], []
    for alloc in nc.m.functions[0].allocations:
        if not isinstance(alloc, mybir.MemoryLocationSet):
            continue
        name = alloc.memorylocations[0].name
        if alloc.kind == "ExternalInput":
            if name != partition_name:
                in_names.append(name)
        elif alloc.kind == "ExternalOutput":
            out_names.append(name)
            shape = tuple(alloc.tensor_shape)
            dtype = mybir.dt.np(alloc.dtype)
            out_avals.append(jax.core.ShapedArray(shape, dtype))
            zero_outs.append(_np.zeros(shape, dtype))
    n_params = len(in_names)
    all_in_names = list(in_names) + list(out_names)
    if partition_name is not None:
        all_in_names.append(partition_name)

    def _body(*args):
        operands = list(args)
        if partition_name is not None:
            operands.append(partition_id_tensor())
        outs = _bass_exec_p.bind(
            *operands,
            out_avals=tuple(out_avals),
            in_names=tuple(all_in_names),
            out_names=tuple(out_names),
            lowering_input_output_aliases=(),
            sim_require_finite=False,
            sim_require_nnan=False,
            nc=nc,
        )
        return tuple(outs)

    donate = tuple(range(n_params, n_params + len(out_avals)))
    devices = jax.devices()[:N_CORES]
    mesh = Mesh(_np.asarray(devices), ("core",))
    in_specs = (PartitionSpec("core"),) * (n_params + len(out_avals))
    out_specs = (PartitionSpec("core"),) * len(out_names)
    fn = jax.jit(shard_map(_body, mesh=mesh, in_specs=in_specs,
                           out_specs=out_specs, check_rep=False),
                 donate_argnums=donate, keep_unused=True)
    ins = []
    for nm in in_names:
        cat = _np.concatenate([_np.asarray(m[nm]) for m in in_maps], axis=0)
        ins.append(jax.device_put(cat, jax.sharding.NamedSharding(mesh, PartitionSpec("core"))))
    zouts = []
    for z in zero_outs:
        cat = _np.concatenate([z] * N_CORES, axis=0)
        zouts.append(jax.device_put(cat, jax.sharding.NamedSharding(mesh, PartitionSpec("core"))))
    outs = fn(*ins, *zouts)
    outs = [_np.asarray(o) for o in outs]
    per_core = []
    for c in range(N_CORES):
        d = {}
        for i, nm in enumerate(out_names):
            full = outs[i]
            rows = full.shape[0] // N_CORES
            d[nm] = full[c * rows:(c + 1) * rows]
        per_core.append(d)
    return per_core


_CACHE = {}


def kernel(node_features, edge_index, edge_features,
           Wl, bl, Wr, br, We, be, attn_vector, Wv, bv, Wo, bo):
    host, per_core = _host_prepare(node_features, edge_index, edge_features,
                                   Wl, bl, Wr, br, We, be, attn_vector, Wv, bv)
    D = host["D"]
    if D not in _CACHE:
        _CACHE[D] = _build_nc(D)
    nc = _CACHE[D]

    Wo_b16 = np.asarray(Wo, dtype=np.float32).astype(bf16)
    bo_f = np.asarray(bo, dtype=np.float32).reshape(128, 1)
    in_maps = []
    for c in range(N_CORES):
        pc = per_core[c]
        in_maps.append({
            "nfT": pc["nfT"], "ohE": pc["ohE"], "ohT": pc["ohT"],
            "efT": pc["efT"], "src16": pc["src16"],
            "Wrv": pc["Wrv"], "Wla": pc["Wla"], "Wea": pc["Wea"],
            "attn": pc["attn"], "Wo": Wo_b16, "bo": bo_f,
        })
    res = _run_spmd(nc, in_maps)
    outT = np.concatenate([res[c]["outT"] for c in range(N_CORES)], axis=1)  # [128, NPAD]
    out_relab = outT.T  # [NPAD, 128]
    out = out_relab[host["new_id"][:N_NODES]]
    return np.ascontiguousarray(out, dtype=np.float32)



# revision 3
# speedup vs baseline: 3.5578x; 3.5578x over previous
"""GATv2 layer on 8 Trainium2 NeuronCores (Bass/Tile).

Strategy (target-major, fully static SPMD):
  * Host relabels nodes (degree-snake) so each 128-node window has ~equal
    incoming-edge mass, sorts edges by (relabeled) target, groups them into
    fixed 128-node windows, and splits each window's edges by source half
    (node id < NH) so gather indices fit int16 for dma_gather.
  * Node projections (left/right/values) are computed on device, sharded
    over cores (each core projects its 6272-node slice); the packed
    right||values table is AllGathered; `left` stays core-local in SBUF.
  * Per window: two dma_gather calls fetch right||values rows per edge;
    `left[target]` is expanded via one-hot matmul from the SBUF left table;
    scores/softmax-numerator/denominator are computed per edge and
    segment-reduced into PSUM via one-hot matmuls (all edges of a target
    live in one window => no cross-core reduction needed).
  * Output = (num/den) @ Wo + bo, written transposed; host undoes layout.

One-hot matrices are precomputed on host and streamed (HWDGE sequential) —
cheaper than descriptor-bound compute paths on device.
"""
import numpy as np
import ml_dtypes

N_CORES = 8
N_NODES = 50000
NPAD = 50176          # 8 * 6272
PER_CORE = NPAD // N_CORES   # 6272
WIN = 128             # nodes per window
W_PER_CORE = PER_CORE // WIN  # 49
W_GLOBAL = NPAD // WIN        # 392
NH = NPAD // 2        # half-table rows (25088 < 32768 -> int16 ok)
IN_F = 256
OUT_F = 128
E_F = 64
H = 8
HD = 16
NEG_SLOPE = 0.2
bf16 = ml_dtypes.bfloat16


# ----------------------------------------------------------------------------
# host-side graph restructuring
# ----------------------------------------------------------------------------

def _host_prepare(node_features, edge_index, edge_features, Wl, bl, Wr, br,
                  We, be, attn_vector, Wv, bv):
    s = np.asarray(edge_index[0], dtype=np.int64)
    t = np.asarray(edge_index[1], dtype=np.int64)
    E = s.shape[0]

    # --- degree-snake relabeling: balance incoming-edge mass per 128-window
    deg = np.bincount(t, minlength=NPAD).astype(np.int64)
    order = np.argsort(-deg, kind="stable")  # all NPAD ids (pads have deg 0)
    order = order[order < N_NODES] if N_NODES < NPAD else order
    # snake across W_GLOBAL windows
    new_id = np.empty(NPAD, dtype=np.int64)
    filln = np.zeros(W_GLOBAL, dtype=np.int64)
    w_seq = np.arange(len(order)) % (2 * W_GLOBAL)
    w_seq = np.where(w_seq < W_GLOBAL, w_seq, 2 * W_GLOBAL - 1 - w_seq)
    for node, w in zip(order, w_seq):
        new_id[node] = w * WIN + filln[w]
        filln[w] += 1
    # pad ids fill remaining slots
    spare = []
    for w in range(W_GLOBAL):
        for k in range(filln[w], WIN):
            spare.append(w * WIN + k)
    spare = np.array(spare, dtype=np.int64)
    pad_nodes = np.arange(N_NODES, NPAD)
    new_id[pad_nodes] = spare[:len(pad_nodes)] if len(pad_nodes) else spare[:0]
    # note: if fewer spare than pads something is off
    inv_id = np.empty(NPAD, dtype=np.int64)
    inv_id[new_id] = np.arange(NPAD)

    ns = new_id[s]
    nt = new_id[t]

    # --- group edges by (window, source-half)
    w_of_edge = nt // WIN
    h_of_edge = (ns >= NH).astype(np.int64)
    key = w_of_edge * 2 + h_of_edge
    eorder = np.argsort(key, kind="stable")
    key_s = key[eorder]
    # counts per (window, half)
    cnt = np.bincount(key_s, minlength=2 * W_GLOBAL).reshape(W_GLOBAL, 2)
    D = int(np.ceil(cnt.max() / WIN))
    NI = D * WIN                 # slots per half
    SLOTS_W = 2 * NI             # slots per window
    EPC = W_PER_CORE * SLOTS_W   # padded edge slots per core

    # slot assignment for each sorted edge
    starts = np.zeros(2 * W_GLOBAL + 1, dtype=np.int64)
    np.cumsum(cnt.reshape(-1), out=starts[1:])
    within = np.arange(E, dtype=np.int64) - starts[key_s]
    slot_global = key_s // 2 * SLOTS_W + (key_s % 2) * NI + within

    # staged per-slot arrays (global, then reshaped per core)
    TOT = W_GLOBAL * SLOTS_W
    tlw_slot = np.full(TOT, 200, dtype=np.int32)
    src_slot = np.zeros(TOT, dtype=np.int64)
    ef_slot = np.zeros((TOT, E_F), dtype=np.float32)
    es = eorder
    tlw_slot[slot_global] = (nt[es] % WIN).astype(np.int32)
    src_slot[slot_global] = ns[es] % NH
    ef_slot[slot_global] = np.asarray(edge_features, dtype=np.float32)[es]

    C = 2 * D  # chunks per window

    # one-hot streams, layout [W_GLOBAL, 128(part), C*128] partition-contiguous
    tlw_wcp = tlw_slot.reshape(W_GLOBAL, C, WIN)  # [w, chunk, pos]
    n_ar = np.arange(WIN, dtype=np.int32)
    # onehotE[w, p, c, n] = (tlw[w, c, p] == n)
    ohE = (tlw_wcp.transpose(0, 2, 1)[:, :, :, None] == n_ar[None, None, None, :])
    ohE = ohE.astype(bf16).reshape(W_GLOBAL, WIN, C * WIN)
    # onehotT[w, n, c, e] = (tlw[w, c, e] == n)
    ohT = (n_ar[None, :, None, None] == tlw_wcp[:, None, :, :])
    ohT = ohT.astype(bf16).reshape(W_GLOBAL, WIN, C * WIN)

    # edge features transposed + ones row: [W_GLOBAL, 65, C*128]
    efT = np.concatenate([ef_slot, np.ones((TOT, 1), np.float32)], axis=1)
    efT = efT.reshape(W_GLOBAL, C * WIN, E_F + 1).transpose(0, 2, 1)
    efT = np.ascontiguousarray(efT, dtype=np.float32).astype(bf16)

    # int16 gather indices, wrapped in 16 partitions replicated x8:
    # position i in a half -> idxs[[i%16, i//16]]
    src_wh = src_slot.reshape(W_GLOBAL, 2, NI)
    wrap = src_wh.reshape(W_GLOBAL, 2, NI // 16, 16).transpose(0, 1, 3, 2)
    wrap = wrap.reshape(W_GLOBAL, 2, 16, NI // 16).astype(np.int16)
    src16 = np.tile(wrap, (1, 1, 8, 1)).reshape(W_GLOBAL, 2, 128, NI // 16)
    src16 = np.ascontiguousarray(src16.transpose(0, 2, 1, 3)).reshape(
        W_GLOBAL, 128, 2 * (NI // 16))

    # node features (relabeled, transposed, +ones row, padded to 384 rows)
    nf = np.zeros((NPAD, IN_F), dtype=np.float32)
    nf[new_id[:N_NODES]] = np.asarray(node_features, dtype=np.float32)
    nfT = np.zeros((384, NPAD), dtype=np.float32)
    nfT[:IN_F] = nf.T
    nfT[IN_F] = 1.0
    nfT = nfT.astype(bf16)

    # weights
    def aug(Wm, bv_):
        a = np.zeros((384, Wm.shape[1]), dtype=np.float32)
        a[:IN_F] = np.asarray(Wm, dtype=np.float32)
        a[IN_F] = np.asarray(bv_, dtype=np.float32)
        return a
    Wrv = np.concatenate([aug(Wr, br), aug(Wv, bv)], axis=1).astype(bf16)  # [384, 256]
    Wla = aug(Wl, bl).astype(bf16)                                          # [384, 128]
    Wea = np.zeros((E_F + 1, H), dtype=np.float32)
    Wea[:E_F] = np.asarray(We, dtype=np.float32)
    Wea[E_F] = np.asarray(be, dtype=np.float32)
    Wea = Wea.astype(bf16)
    attn_flat = np.asarray(attn_vector, dtype=np.float32).reshape(-1)  # [128]
    attn_mat = np.tile(attn_flat[None, :], (128, C)).astype(bf16)

    host = dict(D=D, NI=NI, C=C, EPC=EPC, inv_id=inv_id, new_id=new_id)
    per_core = []
    for c in range(N_CORES):
        wlo, whi = c * W_PER_CORE, (c + 1) * W_PER_CORE
        per_core.append({
            "nfT": np.ascontiguousarray(nfT[:, c * PER_CORE:(c + 1) * PER_CORE]),
            "ohE": np.ascontiguousarray(ohE[wlo:whi]),
            "ohT": np.ascontiguousarray(ohT[wlo:whi]),
            "efT": np.ascontiguousarray(efT[wlo:whi]),
            "src16": np.ascontiguousarray(src16[wlo:whi]),
            "Wrv": Wrv, "Wla": Wla, "Wea": Wea, "attn": attn_mat,
        })
    return host, per_core


# ----------------------------------------------------------------------------
# device kernel
# ----------------------------------------------------------------------------

def _build_nc(D):
    import concourse.bass as bass
    import concourse.bacc as bacc
    import concourse.tile as tile
    from concourse import mybir
    from concourse.masks import make_identity

    f32 = mybir.dt.float32
    b16 = mybir.dt.bfloat16
    i16 = mybir.dt.int16
    NI = D * WIN
    C = 2 * D
    CW = C * WIN

    import os as _os
    _sim1 = bool(_os.environ.get("SIM_1CORE"))
    nc = bacc.Bacc("TRN2", num_devices=(1 if _sim1 else N_CORES), debug=False)
    d_nfT = nc.dram_tensor("nfT", [384, PER_CORE], b16, kind="ExternalInput").ap()
    d_ohE = nc.dram_tensor("ohE", [W_PER_CORE, 128, CW], b16, kind="ExternalInput").ap()
    d_ohT = nc.dram_tensor("ohT", [W_PER_CORE, 128, CW], b16, kind="ExternalInput").ap()
    d_efT = nc.dram_tensor("efT", [W_PER_CORE, E_F + 1, CW], b16, kind="ExternalInput").ap()
    d_src = nc.dram_tensor("src16", [W_PER_CORE, 128, 2 * (NI // 16)], i16, kind="ExternalInput").ap()
    d_Wrv = nc.dram_tensor("Wrv", [384, 256], b16, kind="ExternalInput").ap()
    d_Wla = nc.dram_tensor("Wla", [384, 128], b16, kind="ExternalInput").ap()
    d_Wea = nc.dram_tensor("Wea", [E_F + 1, H], b16, kind="ExternalInput").ap()
    d_attn = nc.dram_tensor("attn", [128, CW], b16, kind="ExternalInput").ap()
    d_Wo = nc.dram_tensor("Wo", [128, 128], b16, kind="ExternalInput").ap()
    d_bo = nc.dram_tensor("bo", [128, 1], f32, kind="ExternalInput").ap()
    d_out = nc.dram_tensor("outT", [128, PER_CORE], f32, kind="ExternalOutput").ap()

    with tile.TileContext(nc) as tc:
        with (
            tc.tile_pool(name="const", bufs=1) as cons,
            tc.tile_pool(name="tbl", bufs=3) as tblp,
            tc.tile_pool(name="win", bufs=4) as winp,
            tc.tile_pool(name="psum", bufs=2, space="PSUM") as psp,
            tc.tile_pool(name="dram", bufs=1, space="DRAM") as dram,
        ):
            # ---- constants
            Wrv_sb = cons.tile([128, 3, 256], b16)
            nc.sync.dma_start(out=Wrv_sb[:], in_=d_Wrv.rearrange("(j p) n -> p j n", p=128))
            Wla_sb = cons.tile([128, 3, 128], b16)
            nc.sync.dma_start(out=Wla_sb[:], in_=d_Wla.rearrange("(j p) n -> p j n", p=128))
            Wea_sb = cons.tile([E_F + 1, H], b16)
            nc.sync.dma_start(out=Wea_sb[:], in_=d_Wea[:, :])
            attn_sb = cons.tile([128, CW], b16)
            nc.sync.dma_start(out=attn_sb[:], in_=d_attn[:, :])
            Wo_sb = cons.tile([128, 128], b16)
            nc.sync.dma_start(out=Wo_sb[:], in_=d_Wo[:, :])
            bo_sb = cons.tile([128, 1], f32)
            nc.sync.dma_start(out=bo_sb[:], in_=d_bo[:, :])
            ident = cons.tile([128, 128], b16)
            make_identity(nc, ident[:])
            left_tab = cons.tile([128, W_PER_CORE * 128], b16)

            # ---- table phase: project this core's node slice
            # (KREPS>1 replicates the whole kernel body for slope-based timing)
            import os
            _kreps = int(os.environ.get("KREPS", "1"))
            rv_loc = dram.tile([PER_CORE, 256], b16)
            rv_full = dram.tile([NPAD, 256], b16)
          # replication loop (timing only; KREPS=1 in production)
          # fmt: off
            for _rep in range(_kreps):
              for tti in range(W_PER_CORE):
                nf3 = tblp.tile([128, 3, 128], b16, tag="nf3")
                nc.sync.dma_start(
                    out=nf3[:],
                    in_=d_nfT.rearrange("(j p) n -> p j n", p=128)[:, :, tti * 128:(tti + 1) * 128])
                ps_rv = psp.tile([128, 256], f32, tag="pA")
                ps_l = psp.tile([128, 128], f32, tag="pB")
                for j in range(3):
                    nc.tensor.matmul(out=ps_rv[:], lhsT=nf3[:, j, :], rhs=Wrv_sb[:, j, :],
                                     start=(j == 0), stop=(j == 2))
                for j in range(3):
                    nc.tensor.matmul(out=ps_l[:], lhsT=nf3[:, j, :], rhs=Wla_sb[:, j, :],
                                     start=(j == 0), stop=(j == 2))
                rv_sb = tblp.tile([128, 256], b16, tag="rvsb")
                nc.vector.tensor_copy(out=rv_sb[:], in_=ps_rv[:])
                nc.vector.tensor_copy(out=left_tab[:, tti * 128:(tti + 1) * 128], in_=ps_l[:])
                nc.sync.dma_start(out=rv_loc[tti * 128:(tti + 1) * 128, :], in_=rv_sb[:])

            if _sim1:
                nc.sync.dma_start(out=rv_full[:PER_CORE, :], in_=rv_loc[:])
            else:
                nc.gpsimd.collective_compute(
                    "AllGather", mybir.AluOpType.bypass,
                    replica_groups=[list(range(N_CORES))],
                    ins=[rv_loc[:].opt()], outs=[rv_full[:].opt()],
                )

            # ---- edge phase
            import os
            _kreps = int(os.environ.get("KREPS", "1"))
            for _rep in range(_kreps):
              for w in range(W_PER_CORE):
                ohE_sb = winp.tile([128, CW], b16, tag="ohE")
                nc.sync.dma_start(out=ohE_sb[:], in_=d_ohE[w, :, :])
                ohT_sb = winp.tile([128, CW], b16, tag="ohT")
                nc.sync.dma_start(out=ohT_sb[:], in_=d_ohT[w, :, :])
                efT_sb = winp.tile([E_F + 1, CW], b16, tag="efT")
                nc.sync.dma_start(out=efT_sb[:], in_=d_efT[w, :, :])
                src_sb = winp.tile([128, 2 * (NI // 16)], i16, tag="src")
                nc.sync.dma_start(out=src_sb[:], in_=d_src[w, :, :])

                rv_g = winp.tile([128, C, 256], b16, tag="rvg")
                nc.gpsimd.dma_gather(
                    out_ap=rv_g[:, :D, :], in_ap=rv_full[:NH, :],
                    idxs_ap=src_sb[:, :NI // 16],
                    num_idxs=NI, num_idxs_reg=NI, elem_size=256, single_packet=False)
                nc.gpsimd.dma_gather(
                    out_ap=rv_g[:, D:, :], in_ap=rv_full[NH:, :],
                    idxs_ap=src_sb[:, NI // 16:],
                    num_idxs=NI, num_idxs_reg=NI, elem_size=256, single_packet=False)

                act_win = winp.tile([128, C, 128], b16, tag="act")
                ps_bias = psp.tile([128, C * H], f32, tag="pB")
                for cc in range(C):
                    ps_comb = psp.tile([128, 128], f32, tag="pA")
                    nc.tensor.matmul(out=ps_comb[:], lhsT=ohT_sb[:, cc * 128:(cc + 1) * 128],
                                     rhs=left_tab[:, w * 128:(w + 1) * 128],
                                     start=True, stop=False)
                    nc.tensor.matmul(out=ps_comb[:], lhsT=ident[:], rhs=rv_g[:, cc, 0:128],
                                     start=False, stop=True)
                    nc.scalar.activation(out=act_win[:, cc, :], in_=ps_comb[:],
                                         func=mybir.ActivationFunctionType.Lrelu,
                                         alpha=NEG_SLOPE)
                    nc.tensor.matmul(out=ps_bias[:, cc * H:(cc + 1) * H],
                                     lhsT=efT_sb[:, cc * 128:(cc + 1) * 128],
                                     rhs=Wea_sb[:], start=True, stop=True)

                prod = winp.tile([128, C, 128], b16, tag="prod")
                nc.vector.tensor_tensor(
                    out=prod[:].rearrange("p c f -> p (c f)"), in0=act_win[:].rearrange("p c f -> p (c f)"),
                    in1=attn_sb[:], op=mybir.AluOpType.mult)
                scores = winp.tile([128, C * H], f32, tag="scores")
                nc.vector.tensor_reduce(
                    out=scores[:], in_=prod[:].rearrange("p c (h d) -> p c h d", h=H),
                    axis=mybir.AxisListType.X, op=mybir.AluOpType.add)
                scores2 = winp.tile([128, C * H], f32, tag="scores2")
                nc.vector.tensor_tensor(out=scores2[:], in0=scores[:], in1=ps_bias[:],
                                        op=mybir.AluOpType.add)
                exp_sb = winp.tile([128, C, H], b16, tag="exp")
                nc.scalar.activation(out=exp_sb[:], in_=scores2[:].rearrange("p (c h) -> p c h", h=H),
                                     func=mybir.ActivationFunctionType.Exp)

                wgt = winp.tile([128, C, 136], b16, tag="wgt")
                nc.vector.tensor_copy(out=wgt[:, :, 128:136], in_=exp_sb[:])
                exp_ap = exp_sb[:]
                nc.vector.tensor_tensor(
                    out=wgt[:, :, 0:128].rearrange("p c (h d) -> p c h d", h=H),
                    in0=rv_g[:, :, 128:256].rearrange("p c (h d) -> p c h d", h=H),
                    in1=bass.AP(tensor=exp_ap.tensor, offset=exp_ap.offset,
                                ap=[[exp_ap.ap[0][0], 128],
                                    [exp_ap.ap[1][0], C], [1, H], [0, HD]]),
                    op=mybir.AluOpType.mult)

                ps_agg = psp.tile([128, 136], f32, tag="pC")
                for cc in range(C):
                    nc.tensor.matmul(out=ps_agg[:], lhsT=ohE_sb[:, cc * 128:(cc + 1) * 128],
                                     rhs=wgt[:, cc, :], start=(cc == 0), stop=(cc == C - 1))

                # ---- finalize window: out = (num/den) @ Wo + bo (transposed)
                den = winp.tile([128, H], f32, tag="den")
                nc.vector.tensor_scalar_add(out=den[:], in0=ps_agg[:, 128:136], scalar1=1e-10)
                rec = winp.tile([128, H], f32, tag="rec")
                nc.vector.reciprocal(out=rec[:], in_=den[:])
                h_sb = winp.tile([128, 128], b16, tag="hsb")
                rec_ap = rec[:]
                nc.vector.tensor_tensor(
                    out=h_sb[:].rearrange("p (h d) -> p h d", h=H),
                    in0=ps_agg[:, 0:128].rearrange("p (h d) -> p h d", h=H),
                    in1=bass.AP(tensor=rec_ap.tensor, offset=rec_ap.offset,
                                ap=[[rec_ap.ap[0][0], 128], [1, H], [0, HD]]),
                    op=mybir.AluOpType.mult)
                ps_T = psp.tile([128, 128], b16, tag="pD")
                nc.tensor.transpose(out=ps_T[:], in_=h_sb[:], identity=ident[:])
                hT_sb = winp.tile([128, 128], b16, tag="hTsb")
                nc.vector.tensor_copy(out=hT_sb[:], in_=ps_T[:])
                ps_out = psp.tile([128, 128], f32, tag="pD")
                nc.tensor.matmul(out=ps_out[:], lhsT=Wo_sb[:], rhs=hT_sb[:],
                                 start=True, stop=True)
                out_sb = winp.tile([128, 128], f32, tag="osb")
                nc.scalar.activation(out=out_sb[:], in_=ps_out[:],
                                     func=mybir.ActivationFunctionType.Identity,
                                     bias=bo_sb[:])
                nc.sync.dma_start(out=d_out[:, w * 128:(w + 1) * 128], in_=out_sb[:])
    nc.compile()
    return nc


# ----------------------------------------------------------------------------
# inline SPMD runner (self-contained; mirrors concourse.bass2jax.run_bass_via_pjrt)
# ----------------------------------------------------------------------------

def _run_spmd(nc, in_maps):
    import jax
    import numpy as _np
    from jax.sharding import Mesh, PartitionSpec
    from jax.experimental.shard_map import shard_map
    import concourse.mybir as mybir
    from concourse.bass2jax import install_neuronx_cc_hook, _bass_exec_p, partition_id_tensor

    install_neuronx_cc_hook()
    partition_name = nc.partition_id_tensor.name if nc.partition_id_tensor else None
    in_names, out_names, out_avals, zero_outs = [], [], [], []
    for alloc in nc.m.functions[0].allocations:
        if not isinstance(alloc, mybir.MemoryLocationSet):
            continue
        name = alloc.memorylocations[0].name
        if alloc.kind == "ExternalInput":
            if name != partition_name:
                in_names.append(name)
        elif alloc.kind == "ExternalOutput":
            out_names.append(name)
            shape = tuple(alloc.tensor_shape)
            dtype = mybir.dt.np(alloc.dtype)
            out_avals.append(jax.core.ShapedArray(shape, dtype))
            zero_outs.append(_np.zeros(shape, dtype))
    n_params = len(in_names)
    all_in_names = list(in_names) + list(out_names)
    if partition_name is not None:
        all_in_names.append(partition_name)

    def _body(*args):
        operands = list(args)
        if partition_name is not None:
            operands.append(partition_id_tensor())
        outs = _bass_exec_p.bind(
            *operands,
            out_avals=tuple(out_avals),
            in_names=tuple(all_in_names),
            out_names=tuple(out_names),
            lowering_input_output_aliases=(),
            sim_require_finite=False,
            sim_require_nnan=False,
            nc=nc,
        )
        return tuple(outs)

    donate = tuple(range(n_params, n_params + len(out_avals)))
    devices = jax.devices()[:N_CORES]
    mesh = Mesh(_np.asarray(devices), ("core",))
    in_specs = (PartitionSpec("core"),) * (n_params + len(out_avals))
    out_specs = (PartitionSpec("core"),) * len(out_names)
    fn = jax.jit(shard_map(_body, mesh=mesh, in_specs=in_specs,
                           out_specs=out_specs, check_rep=False),
                 donate_argnums=donate, keep_unused=True)
    ins = []
    for nm in in_names:
        cat = _np.concatenate([_np.asarray(m[nm]) for m in in_maps], axis=0)
        ins.append(jax.device_put(cat, jax.sharding.NamedSharding(mesh, PartitionSpec("core"))))
    zouts = []
    for z in zero_outs:
        cat = _np.concatenate([z] * N_CORES, axis=0)
        zouts.append(jax.device_put(cat, jax.sharding.NamedSharding(mesh, PartitionSpec("core"))))
    outs = fn(*ins, *zouts)
    outs = [_np.asarray(o) for o in outs]
    per_core = []
    for c in range(N_CORES):
        d = {}
        for i, nm in enumerate(out_names):
            full = outs[i]
            rows = full.shape[0] // N_CORES
            d[nm] = full[c * rows:(c + 1) * rows]
        per_core.append(d)
    return per_core


_CACHE = {}


def kernel(node_features, edge_index, edge_features,
           Wl, bl, Wr, br, We, be, attn_vector, Wv, bv, Wo, bo):
    host, per_core = _host_prepare(node_features, edge_index, edge_features,
                                   Wl, bl, Wr, br, We, be, attn_vector, Wv, bv)
    D = host["D"]
    if D not in _CACHE:
        _CACHE[D] = _build_nc(D)
    nc = _CACHE[D]

    Wo_b16 = np.asarray(Wo, dtype=np.float32).astype(bf16)
    bo_f = np.asarray(bo, dtype=np.float32).reshape(128, 1)
    in_maps = []
    for c in range(N_CORES):
        pc = per_core[c]
        in_maps.append({
            "nfT": pc["nfT"], "ohE": pc["ohE"], "ohT": pc["ohT"],
            "efT": pc["efT"], "src16": pc["src16"],
            "Wrv": pc["Wrv"], "Wla": pc["Wla"], "Wea": pc["Wea"],
            "attn": pc["attn"], "Wo": Wo_b16, "bo": bo_f,
        })
    res = _run_spmd(nc, in_maps)
    outT = np.concatenate([res[c]["outT"] for c in range(N_CORES)], axis=1)  # [128, NPAD]
    out_relab = outT.T  # [NPAD, 128]
    out = out_relab[host["new_id"][:N_NODES]]
    return np.ascontiguousarray(out, dtype=np.float32)



# revision 5
# speedup vs baseline: 3.8465x; 1.0812x over previous
"""GATv2 layer on 8 Trainium2 NeuronCores (Bass/Tile) — v3.

Differences from v1 (baseline):
  * One plain full-row (512B) dma_gather per source-half per 4-window group,
    spread over 4 SWDGE queues (random 512B reads are latency-bound; queue
    parallelism ~1.7x, and plain beats transposed gathers ~4x on HW).
  * Scores computed in head-dim-major space: combinedT[hd,e] accumulates in
    PSUM as (left_win as lhsT) @ ohT  +  (right-chunk as lhsT) @ identity —
    the second matmul transposes the edge-major gather on the fly with f32
    accumulation. PReLU (same ACT table as Exp/Identity -> no table reloads)
    batched per PSUM bank; the attention dot is per-chunk PE matmuls against
    a block-diagonal attn matrix (no DVE mult+reduce, no streamed attn).
  * One-hots stream from host as fp8_e4m3 (exact 0/1, half the bytes of
    bf16) packed with the host-precomputed edge bias (edge_features @ We +
    be) into one per-window DRAM blob; 2 windows per HWDGE DMA.
  * Weighted values multiply reads the gather tile directly (edge-major).
"""
import numpy as np
import ml_dtypes

N_CORES = 8
N_NODES = 50000
NPAD = 50176          # 8 * 6272
PER_CORE = NPAD // N_CORES   # 6272
WIN = 128             # nodes per window
W_PER_CORE = PER_CORE // WIN  # 49
W_GLOBAL = NPAD // WIN        # 392
NH = NPAD // 2        # half-table rows (25088 < 32768 -> int16 ok)
IN_F = 256
OUT_F = 128
E_F = 64
H = 8
HD = 16
NEG_SLOPE = 0.2
GWIN = 4              # windows per gather group
bf16 = ml_dtypes.bfloat16
f8e4 = ml_dtypes.float8_e4m3


# ----------------------------------------------------------------------------
# host-side graph restructuring
# ----------------------------------------------------------------------------

def _host_prepare(node_features, edge_index, edge_features, Wl, bl, Wr, br,
                  We, be, attn_vector, Wv, bv):
    s = np.asarray(edge_index[0], dtype=np.int64)
    t = np.asarray(edge_index[1], dtype=np.int64)
    E = s.shape[0]

    # --- degree-snake relabeling: balance incoming-edge mass per 128-window
    deg = np.bincount(t, minlength=NPAD).astype(np.int64)
    order = np.argsort(-deg, kind="stable")
    order = order[order < N_NODES] if N_NODES < NPAD else order
    new_id = np.empty(NPAD, dtype=np.int64)
    filln = np.zeros(W_GLOBAL, dtype=np.int64)
    w_seq = np.arange(len(order)) % (2 * W_GLOBAL)
    w_seq = np.where(w_seq < W_GLOBAL, w_seq, 2 * W_GLOBAL - 1 - w_seq)
    for node, w in zip(order, w_seq):
        new_id[node] = w * WIN + filln[w]
        filln[w] += 1
    spare = []
    for w in range(W_GLOBAL):
        for k in range(filln[w], WIN):
            spare.append(w * WIN + k)
    spare = np.array(spare, dtype=np.int64)
    pad_nodes = np.arange(N_NODES, NPAD)
    new_id[pad_nodes] = spare[:len(pad_nodes)] if len(pad_nodes) else spare[:0]

    ns = new_id[s]
    nt = new_id[t]

    # --- group edges by (window, source-half), sorted by target within
    w_of_edge = nt // WIN
    h_of_edge = (ns >= NH).astype(np.int64)
    key = (w_of_edge * 2 + h_of_edge) * WIN + (nt % WIN)
    eorder = np.argsort(key, kind="stable")
    key_wh = (w_of_edge * 2 + h_of_edge)[eorder]
    cnt = np.bincount(key_wh, minlength=2 * W_GLOBAL).reshape(W_GLOBAL, 2)
    D = int(np.ceil(cnt.max() / WIN))
    NIH = D * WIN                # slots per half per window
    C = 2 * D                    # chunks per window
    CW = C * WIN                 # slots per window

    starts = np.zeros(2 * W_GLOBAL + 1, dtype=np.int64)
    np.cumsum(cnt.reshape(-1), out=starts[1:])
    within = np.arange(E, dtype=np.int64) - starts[key_wh]
    # slot index within the full [W_GLOBAL, 2, NIH] array
    slot_global = (key_wh // 2) * CW + (key_wh % 2) * NIH + within

    TOT = W_GLOBAL * CW
    tlw_slot = np.full(TOT, 200, dtype=np.int32)     # pad sentinel
    src_slot = np.zeros(TOT, dtype=np.int64)
    ebias_slot = np.zeros((TOT, H), dtype=np.float32)
    es = eorder
    tlw_slot[slot_global] = (nt[es] % WIN).astype(np.int32)
    src_slot[slot_global] = ns[es] % NH

    ebias_all = (np.asarray(edge_features, dtype=np.float32) @
                 np.asarray(We, dtype=np.float32) +
                 np.asarray(be, dtype=np.float32))   # [E, H]
    ebias_slot[slot_global] = ebias_all[es]

    # --- per-window one-hots (fp8) + ebias (bf16) packed into one blob
    tlw_w = tlw_slot.reshape(W_GLOBAL, CW)           # slot order (half, j)
    n_ar = np.arange(WIN, dtype=np.int32)
    ONE8 = np.uint8(0x38)                            # fp8_e4m3 encoding of 1.0
    # ohT[w, n, slot] = (tlw[w, slot] == n)   node-major
    ohT = (n_ar[None, :, None] == tlw_w[:, None, :]).astype(np.uint8) * ONE8
    ohT = ohT.view(f8e4)
    # ohE[w, e128, c*128+n] = (tlw[w, c*128+e128] == n)  edge-major
    tlw_wcp = tlw_w.reshape(W_GLOBAL, C, WIN)
    ohE = (tlw_wcp.transpose(0, 2, 1)[:, :, :, None] == n_ar[None, None, None, :])
    ohE = (ohE.astype(np.uint8) * ONE8).reshape(W_GLOBAL, WIN, CW).view(f8e4)
    # ebias arranged [w, e128, c*8+h]
    eb = ebias_slot.reshape(W_GLOBAL, C, WIN, H).transpose(0, 2, 1, 3)
    eb = np.ascontiguousarray(eb, dtype=np.float32).astype(bf16)
    eb = eb.reshape(W_GLOBAL, WIN, C * H)

    BLOB = 2 * CW + C * H * 2
    blob = np.empty((W_GLOBAL, 128, BLOB), dtype=np.uint8)
    blob[:, :, 0:CW] = ohT.view(np.uint8)
    blob[:, :, CW:2 * CW] = ohE.view(np.uint8)
    blob[:, :, 2 * CW:] = eb.view(np.uint8)
    blob = blob.view(f8e4)

    # --- gather index stream per group per half (wrapped 16p, replicated x8)
    def wrap16(idx):
        n = idx.shape[0]
        w = idx.reshape(n // 16, 16).T.astype(np.int16)     # [16, n/16]
        return np.tile(w, (8, 1))                            # [128, n/16]

    src_whj = src_slot.reshape(W_GLOBAL, 2, NIH)
    groups = []       # list of (wlo, whi) in PER-CORE window ids
    per_core_groups = []
    for c in range(N_CORES):
        glist = []
        w0 = c * W_PER_CORE
        wbeg = 0
        while wbeg < W_PER_CORE:
            g = min(GWIN, W_PER_CORE - wbeg)
            glist.append((wbeg, wbeg + g))
            wbeg += g
        per_core_groups.append(glist)
    NG = len(per_core_groups[0])
    # max group width (all cores share the same group structure)
    src_byte_cols = 2 * (GWIN * NIH // 16)
    src_arr = np.zeros((N_CORES, NG, 128, src_byte_cols), dtype=np.int16)
    for c in range(N_CORES):
        w0 = c * W_PER_CORE
        for gi, (wa, wb) in enumerate(per_core_groups[c]):
            g = wb - wa
            idx0 = src_whj[w0 + wa:w0 + wb, 0, :].reshape(-1)
            idx1 = src_whj[w0 + wa:w0 + wb, 1, :].reshape(-1)
            src_arr[c, gi, :, :g * NIH // 16] = wrap16(idx0)
            src_arr[c, gi, :, GWIN * NIH // 16:
                    GWIN * NIH // 16 + g * NIH // 16] = wrap16(idx1)

    # --- node features (relabeled, transposed, +ones row, padded to 384 rows)
    nf = np.zeros((NPAD, IN_F), dtype=np.float32)
    nf[new_id[:N_NODES]] = np.asarray(node_features, dtype=np.float32)
    nfT = np.zeros((384, NPAD), dtype=np.float32)
    nfT[:IN_F] = nf.T
    nfT[IN_F] = 1.0
    nfT = nfT.astype(bf16)

    # --- weights
    def aug(Wm, bv_):
        a = np.zeros((384, Wm.shape[1]), dtype=np.float32)
        a[:IN_F] = np.asarray(Wm, dtype=np.float32)
        a[IN_F] = np.asarray(bv_, dtype=np.float32)
        return a
    Wrv = np.concatenate([aug(Wr, br), aug(Wv, bv)], axis=1).astype(bf16)
    Wla = aug(Wl, bl).astype(bf16)
    A_blk = np.zeros((128, H), dtype=np.float32)
    av = np.asarray(attn_vector, dtype=np.float32)
    for h in range(H):
        A_blk[h * HD:(h + 1) * HD, h] = av[h]
    A_blk = A_blk.astype(bf16)

    host = dict(D=D, C=C, NIH=NIH, CW=CW, NG=NG, new_id=new_id,
                groups=per_core_groups[0])
    per_core = []
    for c in range(N_CORES):
        wlo, whi = c * W_PER_CORE, (c + 1) * W_PER_CORE
        per_core.append({
            "nfT": np.ascontiguousarray(nfT[:, c * PER_CORE:(c + 1) * PER_CORE]),
            "blob": np.ascontiguousarray(blob[wlo:whi]),
            "src16": np.ascontiguousarray(src_arr[c]),
            "Wrv": Wrv, "Wla": Wla, "A": A_blk,
        })
    return host, per_core


# ----------------------------------------------------------------------------
# device kernel
# ----------------------------------------------------------------------------

def _build_nc(D):
    import concourse.bass as bass
    import concourse.bacc as bacc
    import concourse.tile as tile
    from concourse import mybir
    from concourse.masks import make_identity

    f32 = mybir.dt.float32
    b16 = mybir.dt.bfloat16
    i16 = mybir.dt.int16
    fp8 = mybir.dt.float8e4
    NIH = D * WIN
    C = 2 * D
    CW = C * WIN
    BLOB = 2 * CW + C * H * 2          # fp8-typed dram blob cols = bytes
    NGF = W_PER_CORE // GWIN           # full groups
    GROUPS = [(i * GWIN, min((i + 1) * GWIN, W_PER_CORE))
              for i in range(NGF + (1 if W_PER_CORE % GWIN else 0))]
    NG = len(GROUPS)
    SRC_COLS = 2 * (GWIN * NIH // 16)

    # psum bank chunking of one half (NIH columns, <=512 each)
    half_chunks = []
    off = 0
    while off < NIH:
        n = min(512, NIH - off)
        half_chunks.append((off, n))
        off += n

    import os as _os
    _sim1 = bool(_os.environ.get("SIM_1CORE"))
    nc = bacc.Bacc("TRN2", num_devices=(1 if _sim1 else N_CORES), debug=False,
                   num_swdge_queues=4)
    d_nfT = nc.dram_tensor("nfT", [384, PER_CORE], b16, kind="ExternalInput").ap()
    d_blob = nc.dram_tensor("blob", [W_PER_CORE, 128, BLOB], fp8, kind="ExternalInput").ap()
    d_src = nc.dram_tensor("src16", [NG, 128, SRC_COLS], i16, kind="ExternalInput").ap()
    d_Wrv = nc.dram_tensor("Wrv", [384, 256], b16, kind="ExternalInput").ap()
    d_Wla = nc.dram_tensor("Wla", [384, 128], b16, kind="ExternalInput").ap()
    d_A = nc.dram_tensor("A", [128, H], b16, kind="ExternalInput").ap()
    d_Wo = nc.dram_tensor("Wo", [128, 128], b16, kind="ExternalInput").ap()
    d_bo = nc.dram_tensor("bo", [128, 1], f32, kind="ExternalInput").ap()
    d_out = nc.dram_tensor("outT", [128, PER_CORE], f32, kind="ExternalOutput").ap()

    with tile.TileContext(nc) as tc:
        with (
            tc.tile_pool(name="const", bufs=1) as cons,
            tc.tile_pool(name="tbl", bufs=3) as tblp,
            tc.tile_pool(name="gat", bufs=2) as gatp,
            tc.tile_pool(name="win", bufs=3) as winp,
            tc.tile_pool(name="pcomb", bufs=4, space="PSUM") as pcp,
            tc.tile_pool(name="psmall", bufs=1, space="PSUM") as psp,
            tc.tile_pool(name="dram", bufs=1, space="DRAM") as dram,
        ):
            # ---- constants
            Wrv_sb = cons.tile([128, 3, 256], b16)
            nc.sync.dma_start(out=Wrv_sb[:], in_=d_Wrv.rearrange("(j p) n -> p j n", p=128))
            Wla_sb = cons.tile([128, 3, 128], b16)
            nc.sync.dma_start(out=Wla_sb[:], in_=d_Wla.rearrange("(j p) n -> p j n", p=128))
            A_sb = cons.tile([128, H], b16)
            nc.sync.dma_start(out=A_sb[:], in_=d_A[:, :])
            Wo_sb = cons.tile([128, 128], b16)
            nc.sync.dma_start(out=Wo_sb[:], in_=d_Wo[:, :])
            bo_sb = cons.tile([128, 1], f32)
            nc.sync.dma_start(out=bo_sb[:], in_=d_bo[:, :])
            ident = cons.tile([128, 128], b16)
            make_identity(nc, ident[:])
            left_tab = cons.tile([128, W_PER_CORE * 128], b16)

            import os
            _kreps = int(os.environ.get("KREPS", "1"))
            rv_loc = dram.tile([PER_CORE, 256], b16)
            rv_full = dram.tile([NPAD, 256], b16)

            # ---- table phase: project this core's node slice
            nfT_sb = cons.tile([128, 3, PER_CORE], b16)
            # fmt: off
            for _rep in range(_kreps):
              nc.sync.dma_start(out=nfT_sb[:], in_=d_nfT.rearrange("(j p) n -> p j n", p=128))
              for t0 in range(0, W_PER_CORE, 4):
                nt = min(4, W_PER_CORE - t0)
                rv_sb = tblp.tile([128, 4, 256], b16, tag="rvsb")
                for ti in range(nt):
                    tti = t0 + ti
                    ps_rv = pcp.tile([128, 256], f32, tag="pc")
                    ps_l = pcp.tile([128, 128], f32, tag="pc")
                    for j in range(3):
                        nc.tensor.matmul(out=ps_rv[:],
                                         lhsT=nfT_sb[:, j, tti * 128:(tti + 1) * 128],
                                         rhs=Wrv_sb[:, j, :],
                                         start=(j == 0), stop=(j == 2))
                    for j in range(3):
                        nc.tensor.matmul(out=ps_l[:],
                                         lhsT=nfT_sb[:, j, tti * 128:(tti + 1) * 128],
                                         rhs=Wla_sb[:, j, :],
                                         start=(j == 0), stop=(j == 2))
                    nc.vector.tensor_copy(out=rv_sb[:, ti, :], in_=ps_rv[:])
                    nc.vector.tensor_copy(out=left_tab[:, tti * 128:(tti + 1) * 128], in_=ps_l[:])
                nc.sync.dma_start(
                    out=rv_loc[t0 * 128:(t0 + nt) * 128, :]
                        .rearrange("(t p) n -> p t n", p=128),
                    in_=rv_sb[:, :nt, :])

            if _sim1:
                nc.sync.dma_start(out=rv_full[:PER_CORE, :], in_=rv_loc[:])
            else:
                nc.gpsimd.collective_compute(
                    "AllGather", mybir.AluOpType.bypass,
                    replica_groups=[list(range(N_CORES))],
                    ins=[rv_loc[:].opt()], outs=[rv_full[:].opt()],
                )

            # ---- edge phase
            for _rep in range(_kreps):
              for gi, (wa, wb) in enumerate(GROUPS):
                G = wb - wa
                NIg = G * NIH
                src_sb = gatp.tile([128, SRC_COLS], i16, tag="src")
                nc.sync.dma_start(out=src_sb[:], in_=d_src[gi, :, :])

                # plain full-row gather per half: edge-major [e, 256]
                # ([:, :, 0:128]=right, [:, :, 128:256]=values)
                rvg = []
                for hf in range(2):
                    rlo = 0 if hf == 0 else NH
                    idx_ap = src_sb[:, hf * (GWIN * NIH // 16):
                                    hf * (GWIN * NIH // 16) + NIg // 16]
                    r_t = gatp.tile([128, NIg // 128, 256], b16, tag=f"rvg{hf}")
                    nc.gpsimd.dma_gather(
                        out_ap=r_t[:], in_ap=rv_full[rlo:rlo + NH, :],
                        idxs_ap=idx_ap, num_idxs=NIg, num_idxs_reg=NIg,
                        elem_size=256, single_packet=False,
                        queue_num=(2 * gi + hf) % 4)
                    rvg.append(r_t)

                blob2 = {}
                for w0 in range(wa, wb, 2):
                    nb = min(2, wb - w0)
                    bt = winp.tile([128, 2, BLOB], fp8, tag="blob")
                    nc.sync.dma_start(
                        out=bt[:, :nb, :],
                        in_=d_blob[w0:w0 + nb, :, :].rearrange("w p b -> p w b"))
                    for k in range(nb):
                        blob2[w0 + k] = bt[:, k, :]
                for w in range(wa, wb):
                    wg = w - wa    # window index within group
                    blob = blob2[w]
                    ohT = blob[:, 0:CW]
                    ohE = blob[:, CW:2 * CW]
                    ebias = blob[:, 2 * CW:].bitcast(b16)   # [128, C*H]

                    # combinedT[hd, e] = left[tlw]^T + right^T, PReLU per bank.
                    # rightT lands in PSUM via lhsT=right-chunk, rhs=identity
                    # (a matmul-transpose with f32 accumulation).
                    actT = winp.tile([128, CW], b16, tag="actT")
                    for b0 in range(0, C, 4):
                        nb4 = min(4, C - b0)
                        ps_c = pcp.tile([128, 512], f32, tag="pc")
                        for j in range(nb4):
                            cc = b0 + j
                            hf, jj = cc // D, cc % D
                            nc.tensor.matmul(
                                out=ps_c[:, j * 128:(j + 1) * 128],
                                lhsT=left_tab[:, w * 128:(w + 1) * 128],
                                rhs=ohT[:, cc * 128:(cc + 1) * 128],
                                start=(j == 0), stop=False,
                                skip_group_check=True)
                            nc.tensor.matmul(
                                out=ps_c[:, j * 128:(j + 1) * 128],
                                lhsT=rvg[hf][:, wg * D + jj, 0:128],
                                rhs=ident[:],
                                start=False, stop=(j == nb4 - 1),
                                skip_group_check=True)
                        nc.scalar.activation(
                            out=actT[:, b0 * 128:(b0 + nb4) * 128],
                            in_=ps_c[:, :nb4 * 128],
                            func=mybir.ActivationFunctionType.Prelu,
                            alpha=NEG_SLOPE)

                    # scores: ebias + per-chunk attn matmuls -> [128, C*H]
                    ps_sc = psp.tile([128, C * H], f32, tag="pS", bufs=1)
                    nc.tensor.matmul(out=ps_sc[:], lhsT=ident[:], rhs=ebias[:, :],
                                     start=True, stop=False, skip_group_check=True)
                    for cc in range(C):
                        nc.tensor.matmul(
                            out=ps_sc[:, cc * H:(cc + 1) * H],
                            lhsT=actT[:, cc * 128:(cc + 1) * 128],
                            rhs=A_sb[:], start=False, stop=(cc == C - 1),
                            skip_group_check=True)
                    exp_sb = winp.tile([128, C, H], b16, tag="exp")
                    nc.scalar.activation(
                        out=exp_sb[:], in_=ps_sc[:].rearrange("p (c h) -> p c h", h=H),
                        func=mybir.ActivationFunctionType.Exp)

                    # weighted values straight from the edge-major gather
                    wgt = winp.tile([128, C, 136], b16, tag="wgt")
                    nc.vector.tensor_copy(out=wgt[:, :, 128:136], in_=exp_sb[:])
                    for hf in range(2):
                        exp_ap = exp_sb[:, hf * D:(hf + 1) * D, :]
                        nc.vector.tensor_tensor(
                            out=wgt[:, hf * D:(hf + 1) * D, 0:128]
                                .rearrange("p c (h d) -> p c h d", h=H),
                            in0=rvg[hf][:, wg * D:(wg + 1) * D, 128:256]
                                .rearrange("p c (h d) -> p c h d", h=H),
                            in1=bass.AP(tensor=exp_ap.tensor, offset=exp_ap.offset,
                                        ap=[[exp_ap.ap[0][0], 128],
                                            [exp_ap.ap[1][0], D], [1, H], [0, HD]]),
                            op=mybir.AluOpType.mult)

                    # segment-sum via one-hot matmuls
                    ps_agg = psp.tile([128, 136], f32, tag="pG", bufs=2)
                    for cc in range(C):
                        nc.tensor.matmul(out=ps_agg[:],
                                         lhsT=ohE[:, cc * 128:(cc + 1) * 128],
                                         rhs=wgt[:, cc, :],
                                         start=(cc == 0), stop=(cc == C - 1))

                    # finalize: out = (num/den) @ Wo + bo (transposed)
                    den = winp.tile([128, H], f32, tag="den")
                    nc.vector.tensor_scalar_add(out=den[:], in0=ps_agg[:, 128:136],
                                                scalar1=1e-10)
                    rec = winp.tile([128, H], f32, tag="rec")
                    nc.vector.reciprocal(out=rec[:], in_=den[:])
                    h_sb = winp.tile([128, 128], b16, tag="hsb")
                    rec_ap = rec[:]
                    nc.vector.tensor_tensor(
                        out=h_sb[:].rearrange("p (h d) -> p h d", h=H),
                        in0=ps_agg[:, 0:128].rearrange("p (h d) -> p h d", h=H),
                        in1=bass.AP(tensor=rec_ap.tensor, offset=rec_ap.offset,
                                    ap=[[rec_ap.ap[0][0], 128], [1, H], [0, HD]]),
                        op=mybir.AluOpType.mult)
                    ps_T = psp.tile([128, 128], b16, tag="pT")
                    nc.tensor.transpose(out=ps_T[:], in_=h_sb[:], identity=ident[:])
                    hT_sb = winp.tile([128, 128], b16, tag="hTsb")
                    nc.vector.tensor_copy(out=hT_sb[:], in_=ps_T[:])
                    ps_out = psp.tile([128, 128], f32, tag="pT")
                    nc.tensor.matmul(out=ps_out[:], lhsT=Wo_sb[:], rhs=hT_sb[:],
                                     start=True, stop=True)
                    if (w - wa) % 2 == 0:
                        out_st = winp.tile([128, 2, 128], f32, tag="osb")
                    nc.scalar.activation(out=out_st[:, (w - wa) % 2, :], in_=ps_out[:],
                                         func=mybir.ActivationFunctionType.Identity,
                                         bias=bo_sb[:])
                    if (w - wa) % 2 == 1 or w == wb - 1:
                        wlo = w - ((w - wa) % 2)
                        nc.sync.dma_start(
                            out=d_out[:, wlo * 128:(w + 1) * 128],
                            in_=out_st[:, :(w - wlo) + 1, :])
    nc.compile()
    return nc


# ----------------------------------------------------------------------------
# inline SPMD runner (self-contained)
# ----------------------------------------------------------------------------

def _run_spmd(nc, in_maps):
    import jax
    import numpy as _np
    from jax.sharding import Mesh, PartitionSpec
    from jax.experimental.shard_map import shard_map
    import concourse.mybir as mybir
    from concourse.bass2jax import install_neuronx_cc_hook, _bass_exec_p, partition_id_tensor

    install_neuronx_cc_hook()
    partition_name = nc.partition_id_tensor.name if nc.partition_id_tensor else None
    in_names, out_names, out_avals, zero_outs = [], [], [], []
    for alloc in nc.m.functions[0].allocations:
        if not isinstance(alloc, mybir.MemoryLocationSet):
            continue
        name = alloc.memorylocations[0].name
        if alloc.kind == "ExternalInput":
            if name != partition_name:
                in_names.append(name)
        elif alloc.kind == "ExternalOutput":
            out_names.append(name)
            shape = tuple(alloc.tensor_shape)
            dtype = mybir.dt.np(alloc.dtype)
            out_avals.append(jax.core.ShapedArray(shape, dtype))
            zero_outs.append(_np.zeros(shape, dtype))
    n_params = len(in_names)
    all_in_names = list(in_names) + list(out_names)
    if partition_name is not None:
        all_in_names.append(partition_name)

    def _body(*args):
        operands = list(args)
        if partition_name is not None:
            operands.append(partition_id_tensor())
        outs = _bass_exec_p.bind(
            *operands,
            out_avals=tuple(out_avals),
            in_names=tuple(all_in_names),
            out_names=tuple(out_names),
            lowering_input_output_aliases=(),
            sim_require_finite=False,
            sim_require_nnan=False,
            nc=nc,
        )
        return tuple(outs)

    donate = tuple(range(n_params, n_params + len(out_avals)))
    devices = jax.devices()[:N_CORES]
    mesh = Mesh(_np.asarray(devices), ("core",))
    in_specs = (PartitionSpec("core"),) * (n_params + len(out_avals))
    out_specs = (PartitionSpec("core"),) * len(out_names)
    fn = jax.jit(shard_map(_body, mesh=mesh, in_specs=in_specs,
                           out_specs=out_specs, check_rep=False),
                 donate_argnums=donate, keep_unused=True)
    ins = []
    for nm in in_names:
        cat = _np.concatenate([_np.asarray(m[nm]) for m in in_maps], axis=0)
        ins.append(jax.device_put(cat, jax.sharding.NamedSharding(mesh, PartitionSpec("core"))))
    zouts = []
    for z in zero_outs:
        cat = _np.concatenate([z] * N_CORES, axis=0)
        zouts.append(jax.device_put(cat, jax.sharding.NamedSharding(mesh, PartitionSpec("core"))))
    outs = fn(*ins, *zouts)
    outs = [_np.asarray(o) for o in outs]
    per_core = []
    for c in range(N_CORES):
        d = {}
        for i, nm in enumerate(out_names):
            full = outs[i]
            rows = full.shape[0] // N_CORES
            d[nm] = full[c * rows:(c + 1) * rows]
        per_core.append(d)
    return per_core


_CACHE = {}


def kernel(node_features, edge_index, edge_features,
           Wl, bl, Wr, br, We, be, attn_vector, Wv, bv, Wo, bo):
    host, per_core = _host_prepare(node_features, edge_index, edge_features,
                                   Wl, bl, Wr, br, We, be, attn_vector, Wv, bv)
    D = host["D"]
    if D not in _CACHE:
        _CACHE[D] = _build_nc(D)
    nc = _CACHE[D]

    Wo_b16 = np.asarray(Wo, dtype=np.float32).astype(bf16)
    bo_f = np.asarray(bo, dtype=np.float32).reshape(128, 1)
    in_maps = []
    for c in range(N_CORES):
        pc = per_core[c]
        in_maps.append({
            "nfT": pc["nfT"], "blob": pc["blob"], "src16": pc["src16"],
            "Wrv": pc["Wrv"], "Wla": pc["Wla"], "A": pc["A"],
            "Wo": Wo_b16, "bo": bo_f,
        })
    res = _run_spmd(nc, in_maps)
    outT = np.concatenate([res[c]["outT"] for c in range(N_CORES)], axis=1)
    out_relab = outT.T
    out = out_relab[host["new_id"][:N_NODES]]
    return np.ascontiguousarray(out, dtype=np.float32)
